# revision 75
# baseline (speedup 1.0000x reference)
"""Trainium2 Bass kernel for a custom GRU-like cell.

Reference computation (per row n of x, h; all weights 256x256, applied x @ W.T + b):
    z        = sigmoid(x W_z^T + b_Wz + h U_z^T + b_Uz)
    r        = sigmoid(x W_r^T + b_Wr + h U_r^T + b_Ur)
    cand_in  = x W_h^T + b_Wh + (r*h) U_h^T + b_Uh + bias_h
    gate     = sigmoid(cand_in Lg^T + b_g)
    candidate= (cand_in Ll^T + b_l) * gate
    out      = z * candidate + (1 - z) * h

Strategy: data-parallel over rows across 8 cores.  The whole kernel runs in
"feature-major" (transposed) layout [feat, row]: the host ships x^T and h^T
(bf16 + fp8 copies), so the device does zero transposes -- every gemm chains
with weights as the stationary operand.

Key tricks:
 - The gate/linear matrices are composed with the cand matrices on the host
   (Lg@Wh, Lg@Uh, Ll@Wh, Ll@Uh), so g and l are computed straight from x and
   r*h: the cand intermediate (its PSUM round-trip and SBUF copies) never
   exists, at zero extra matmul cost.
 - Gemm stages listed in GRU_FP8 (subset of "zrgl", default "rgl" -- z must
   stay bf16 for accuracy, it multiplies the large (l*g - h) term) run as
   fp8e4 DoubleRow matmuls (2 MACs/cell/cycle, K=256 per instruction); fp8
   weights are pre-scaled x256 into e4m3 normal range and compensated via the
   ACT free-affine scale.
 - Sigmoids+bias run on ACT straight out of PSUM at FD=1024; l's bias-add is
   fused into the combine's first scalar_tensor_tensor (no ACT pass for l).
 - Elementwise tail: rh8 = r*h fused multiply-cast on DVE (bf16 2x), combine
   split DVE (PSUM-reading ops) / GPSIMD (final mul/add) / SWDGE (y store).
 - r is computed before z so the sigma(r) -> r*h -> g/l-gemm chain of the
   next pair overlaps z's matmuls.
 - Bias folding (host side): b_z = b_Wz + b_Uz, b_r likewise; the cand bias
   b_c = b_Wh + b_Uh + bias_h is folded into b_g' = Lg b_c + b_g and
   b_l' = Ll b_c + b_l.
Work is emitted as two-subtile software-pipelined pairs (stage-major within a
pair) so each engine always has independent work in flight; subtiles are 1024
rows (PSUM double-bank per half), with an optional 512-row tail.
"""

import os

import numpy as np

import concourse.bass as bass
import concourse.tile as tile
from concourse import bacc, mybir
from concourse import bass_utils

N_CORES = 8
D = 256
PART = 128
TILE_N = 1024  # rows per subtile
HALF = 512     # fp32 PSUM bank / max matmul free dim

F32 = mybir.dt.float32
BF16 = mybir.dt.bfloat16
FP8 = mybir.dt.float8e4
AF = mybir.ActivationFunctionType
DR = mybir.MatmulPerfMode.DoubleRow

W_SCALE = 256.0  # fp8 weights pre-scaled into e4m3 normal range


def _fp8_cfg():
    return os.environ.get("GRU_FP8", "rgl")


def _build(r_pad: int, loop_reps: int = 1, fp8_cfg: str = "zrgl"):
    """Build and compile the single-core Bass program for r_pad rows."""
    assert r_pad % HALF == 0
    nsub = -(-r_pad // TILE_N)  # last subtile may be a HALF-length tail

    def L_of(t):
        return min(TILE_N, r_pad - t * TILE_N)
    f_z, f_r, f_g, f_l = (k in fp8_cfg for k in "zrgl")
    any_fp8 = f_z or f_r or f_g or f_l
    need_xh8 = any_fp8  # x8 for any fp8 gemm; h8 for fp8 z/r (loaded together)
    pool_lvl = int(os.environ.get("GRU_POOL", "1"))
    pool_cd = pool_lvl >= 1

    nc = bacc.Bacc("TRN2", target_bir_lowering=False, debug=False)

    xt_d = nc.dram_tensor("xt", (D, r_pad), BF16, kind="ExternalInput")
    ht_d = nc.dram_tensor("ht", (D, r_pad), BF16, kind="ExternalInput")
    w_d = nc.dram_tensor("wts", (8, 2, PART, D), BF16, kind="ExternalInput")
    b_d = nc.dram_tensor("biases", (D, 4), F32, kind="ExternalInput")
    y_d = nc.dram_tensor("y", (D, r_pad), BF16, kind="ExternalOutput")
    if need_xh8:
        x8_d = nc.dram_tensor("xt8", (D, r_pad), FP8, kind="ExternalInput")
        h8_d = nc.dram_tensor("ht8", (D, r_pad), FP8, kind="ExternalInput")
    if any_fp8:
        w8_d = nc.dram_tensor("wts8", (8, 2, PART, D), FP8, kind="ExternalInput")

    xv = xt_d.ap().rearrange("(c p) n -> p c n", p=PART)
    hv = ht_d.ap().rearrange("(c p) n -> p c n", p=PART)
    yv = y_d.ap().rearrange("(m p) n -> p m n", p=PART)
    if need_xh8:
        xv8 = x8_d.ap().rearrange("(c p) n -> p c n", p=PART)
        hv8 = h8_d.ap().rearrange("(c p) n -> p c n", p=PART)

    with tile.TileContext(nc) as tc:
        with (
            tc.tile_pool(name="const", bufs=1) as const,
            tc.tile_pool(name="io", bufs=4) as io,
            tc.tile_pool(name="act", bufs=4) as act,
            tc.tile_pool(name="ps", bufs=4, space="PSUM") as ps,
        ):
            w_sb = const.tile([PART, 8, 2, D], BF16, tag="w")
            b_sb = const.tile([PART, 2, 4], F32, tag="b")
            if any_fp8:
                w8_sb = const.tile([PART, 8, 2, D], FP8, tag="w8")

            def emit_consts():
                # fp8 weights + biases first (the r-gemms need them earliest);
                # the big bf16 weight load rides the idle gpsimd queue
                if any_fp8:
                    nc.sync.dma_start(w8_sb[:], w8_d.ap().rearrange("w k p o -> p w k o"))
                nc.sync.dma_start(b_sb[:], b_d.ap().rearrange("(k p) j -> p k j", p=PART))
                nc.gpsimd.dma_start(w_sb[:], w_d.ap().rearrange("w k p o -> p w k o"))

            state = {}

            def mm_group(pmap, wi, ui, rmap, use_fp8, m):
                """Per-m PSUM accumulations W x + U u for each subtile in the
                group, weight-major so each stationary (esp. fp8 DoubleRow,
                which has no fast-weight-load) serves all group matmuls."""
                msl = slice(m * PART, (m + 1) * PART)
                ts = list(pmap.keys())
                if use_fp8:
                    # DoubleRow: contraction pair dim = the k-chunk dim
                    srcs = [(wi, 0)] + ([(ui, 1)] if ui is not None else [])
                    for k, (gi, ri) in enumerate(srcs):
                        for t in ts:
                            for j in range(L_of(t) // HALF):
                                js = slice(j * HALF, (j + 1) * HALF)
                                nc.tensor.matmul(
                                    pmap[t][:, j], w8_sb[:, gi, :, msl],
                                    rmap[t][ri][:, :, js],
                                    start=k == 0, stop=k == len(srcs) - 1,
                                    perf_mode=DR)
                else:
                    srcs = [(wi, 0, 0), (wi, 0, 1)]
                    if ui is not None:
                        srcs += [(ui, 1, 0), (ui, 1, 1)]
                    for k, (gi, ri, c) in enumerate(srcs):
                        for t in ts:
                            for j in range(L_of(t) // HALF):
                                js = slice(j * HALF, (j + 1) * HALF)
                                nc.tensor.matmul(
                                    pmap[t][:, j], w_sb[:, gi, c, msl],
                                    rmap[t][ri][:, c, js],
                                    start=k == 0, stop=k == len(srcs) - 1)

            def emit_input(t, first=False):
                Lt = L_of(t)
                sl = slice(t * TILE_N, t * TILE_N + Lt)
                st = state[t] = {}
                if need_xh8:
                    # fp8 copies first: stage1's r-gemms consume them earliest.
                    # The very first subtile's ride the gpsimd queue so they
                    # load in parallel with w8 on the sync queue (ramp).
                    q8 = nc.gpsimd if first else nc.sync
                    x8_s = io.tile([PART, 2, TILE_N], FP8, tag="x8_s")
                    q8.dma_start(x8_s[:, :, :Lt], xv8[:, :, sl])
                    h8_s = io.tile([PART, 2, TILE_N], FP8, tag="h8_s")
                    q8.dma_start(h8_s[:, :, :Lt], hv8[:, :, sl])
                    st["x8"], st["h8"] = x8_s, h8_s
                x_s = io.tile([PART, 2, TILE_N], BF16, tag="x_s")
                nc.sync.dma_start(x_s[:, :, :Lt], xv[:, :, sl])
                h_s = io.tile([PART, 2, TILE_N], BF16, tag="h_s")
                nc.sync.dma_start(h_s[:, :, :Lt], hv[:, :, sl])
                st["x"], st["h"] = x_s, h_s

            def emit_stage1(grp, gates=((2, 3, "r", 1), (0, 1, "z", 0))):
                for t in grp:
                    st = state[t]
                    for wi, ui, key, bj in gates:
                        use8 = {"r": f_r, "z": f_z}[key]
                        st[key] = act.tile([PART, 2, TILE_N], BF16, tag=key + "_t",
                                           name=key + "_t")
                        rmap = {t: (st["x8"], st["h8"]) if use8 else (st["x"], st["h"])}
                        for m in range(2):
                            pmap = {t: ps.tile([PART, 2, HALF], F32, tag="ps", name="ps")}
                            mm_group(pmap, wi, ui, rmap, use8, m)
                            jn = L_of(t) // HALF
                            nc.scalar.activation(st[key][:, m, : L_of(t)],
                                                 pmap[t][:, :jn],
                                                 AF.Sigmoid,
                                                 bias=b_sb[:, m, bj : bj + 1],
                                                 scale=1.0 / W_SCALE if use8 else 1.0)

            def emit_rh(t):
                st = state[t]
                eng = nc.gpsimd if pool_lvl >= 2 else nc.vector
                w = (slice(None), slice(None), slice(0, L_of(t)))
                if f_g and f_l:
                    # rh8 = r * h straight to fp8 (bf16 rh has no other consumer)
                    rh8 = act.tile([PART, 2, TILE_N], FP8, tag="rh8")
                    eng.tensor_mul(rh8[w], st["r"][w], st["h"][w])
                    st["rh8"] = rh8
                else:
                    # rh = r * h in place over r_t (bf16 2x)
                    eng.tensor_mul(st["r"][w], st["r"][w], st["h"][w])
                    if f_g or f_l:
                        rh8 = act.tile([PART, 2, TILE_N], FP8, tag="rh8")
                        eng.tensor_copy(rh8[w], st["r"][w])
                        st["rh8"] = rh8

            def emit_stage3(grp):
                # composed gemms straight from x and rh: no cand intermediate.
                # g = sigmoid((Lg Wh) x + (Lg Uh) rh + bg'); l likewise with Ll.
                for t in grp:
                    state[t]["g"] = act.tile([PART, 2, TILE_N], BF16, tag="g_t", name="g_t")
                    state[t]["l0"] = []
                for t in grp:
                    st = state[t]
                    gmap = {t: (st["x8"], st["rh8"]) if f_g else (st["x"], st["r"])}
                    lmap = {t: (st["x8"], st["rh8"]) if f_l else (st["x"], st["r"])}
                    # g and l complete per-subtile and per-half so the
                    # combine's first inputs (sigma(g) m0, l0 m0) finish ASAP
                    for m in range(2):
                        pmap = {t: ps.tile([PART, 2, HALF], F32, tag="ps", name="ps")}
                        mm_group(pmap, 4, 5, gmap, f_g, m)
                        jn = L_of(t) // HALF
                        nc.scalar.activation(st["g"][:, m, : L_of(t)],
                                             pmap[t][:, :jn],
                                             AF.Sigmoid, bias=b_sb[:, m, 2:3],
                                             scale=1.0 / W_SCALE if f_g else 1.0)
                        # l stays in PSUM; bias-add fused into the combine STT
                        pmap = {t: ps.tile([PART, 2, HALF], F32, tag="ps", name="ps")}
                        mm_group(pmap, 6, 7, lmap, f_l, m)
                        st["l0"].append(pmap[t])

            AOP = mybir.AluOpType

            def emit_combine(t, last=False):
                st = state[t]
                Lt = L_of(t)
                jn = Lt // HALF
                sl = slice(t * TILE_N, t * TILE_N + Lt)
                w = (slice(None), slice(None), slice(0, Lt))
                h_s = st["h"]
                q_t = act.tile([PART, 2, TILE_N], BF16, tag="q_t")
                # q = (l0 + bl') * g   (bl' pre-scaled by W_SCALE when l is fp8)
                for m in range(2):
                    nc.vector.scalar_tensor_tensor(
                        q_t[:, m, :Lt], st["l0"][m][:, :jn], b_sb[:, m, 3:4],
                        st["g"][:, m, :Lt], op0=AOP.add, op1=AOP.mult)
                # q = q/W_SCALE - h ; y = q*z + h
                # (two ops: tensor_scalar runs 4x and tensor_sub 2x on bf16,
                # while the fused scalar_tensor_tensor only has a 1x uop)
                if f_l:
                    nc.vector.tensor_scalar_mul(q_t[w], q_t[w], 1.0 / W_SCALE)
                nc.vector.tensor_sub(q_t[w], q_t[w], h_s[w])
                eng = nc.gpsimd if (pool_cd and not last) else nc.vector
                eng.tensor_mul(q_t[w], q_t[w], st["z"][w])
                eng.tensor_add(q_t[w], q_t[w], h_s[w])
                nc.sync.dma_start(yv[:, :, sl], q_t[w])
                del state[t]

            def emit_all():
                # G-subtile groups, software-pipelined with a 3-stage skew:
                # iter i emits stage1(group i), stage2(group i-1), stage3+
                # combine(group i-2) so every consumer's producer ran a full
                # iteration earlier and no engine waits on a same-iter chain.
                G = int(os.environ.get("GRU_GROUP", "2"))
                skew = os.environ.get("GRU_SKEW", "0") == "1"
                groups = [tuple(range(t, min(t + G, nsub))) for t in range(0, nsub, G)]
                ng = len(groups)
                if not skew:
                    for i, grp in enumerate(groups):
                        if i == 0:
                            for k, t in enumerate(grp):
                                emit_input(t, first=k == 0)
                        emit_stage1(grp)
                        if i + 1 < ng:
                            for t in groups[i + 1]:
                                emit_input(t)
                        for t in grp:
                            emit_rh(t)
                        emit_stage3(grp)
                        for t in grp:
                            emit_combine(t, last=i >= ng - 2)
                    return
                for i in range(ng + 1):
                    if i == 0:
                        for t in groups[0]:
                            emit_input(t)
                    if i < ng:
                        emit_stage1(groups[i])
                        if i + 1 < ng:
                            for t in groups[i + 1]:
                                emit_input(t)
                    if i >= 1:
                        for t in groups[i - 1]:
                            emit_rh(t)
                        emit_stage3(groups[i - 1])
                        for t in groups[i - 1]:
                            emit_combine(t)

            emit_consts()
            if loop_reps > 1:
                # timing harness only: repeat the whole pass in a HW loop
                with tc.For_i(0, loop_reps, 1, staggered_reset=True, hint_engines=(
                        mybir.EngineType.PE, mybir.EngineType.Activation,
                        mybir.EngineType.DVE, mybir.EngineType.SP,
                        mybir.EngineType.Pool)):
                    emit_all()
            else:
                emit_all()

    nc.compile()
    return nc


_NC_CACHE: dict[tuple, object] = {}


def _get_nc(r_pad: int):
    loop_reps = int(os.environ.get("GRU_LOOP_REPS", "1"))
    cfg = _fp8_cfg()
    key = (r_pad, loop_reps, cfg, os.environ.get("GRU_POOL", "1"), os.environ.get("GRU_GROUP", "2"), os.environ.get("GRU_SKEW", "0"))
    if key not in _NC_CACHE:
        _NC_CACHE[key] = _build(r_pad, loop_reps, cfg)
    return _NC_CACHE[key]


def kernel(x, h,
           W_z_w, W_z_b, U_z_w, U_z_b,
           W_r_w, W_r_b, U_r_w, U_r_b,
           W_h_w, W_h_b, U_h_w, U_h_b,
           lin_gate_w, lin_gate_b, lin_linear_w, lin_linear_b,
           bias_h):
    bf16 = mybir.dt.np(BF16)
    fp8 = mybir.dt.np(FP8)
    cfg = _fp8_cfg()
    need_xh8 = any(k in cfg for k in "zrgl")
    x = np.asarray(x, dtype=np.float32)
    h = np.asarray(h, dtype=np.float32)
    n_rows = x.shape[0]

    # host-side weight prep: transpose weights, fold biases.  The gate/linear
    # matrices are composed with the cand matrices (Lg@Wh etc.) so the device
    # computes g and l straight from x and r*h with no cand intermediate.
    Wh = np.asarray(W_h_w, np.float64)
    Uh = np.asarray(U_h_w, np.float64)
    Lg = np.asarray(lin_gate_w, np.float64)
    Ll = np.asarray(lin_linear_w, np.float64)
    ws = [W_z_w, U_z_w, W_r_w, U_r_w, Lg @ Wh, Lg @ Uh, Ll @ Wh, Ll @ Uh]
    wt = np.stack(
        [np.ascontiguousarray(np.asarray(w, np.float64).astype(np.float32).T).reshape(2, PART, D)
         for w in ws]
    )
    wts = wt.astype(bf16)
    b_c = (np.asarray(W_h_b, np.float64) + np.asarray(U_h_b, np.float64)
           + np.asarray(bias_h, np.float64))
    biases = np.stack(
        [
            np.asarray(W_z_b, np.float64) + np.asarray(U_z_b, np.float64),
            np.asarray(W_r_b, np.float64) + np.asarray(U_r_b, np.float64),
            np.asarray(lin_gate_w, np.float64) @ b_c + np.asarray(lin_gate_b, np.float64),
            (np.asarray(lin_linear_w, np.float64) @ b_c + np.asarray(lin_linear_b, np.float64))
            * (W_SCALE if "l" in cfg else 1.0),
        ],
        axis=1,
    ).astype(np.float32)

    # shard rows across cores, pad each shard to a multiple of TILE_N
    per = (n_rows + N_CORES - 1) // N_CORES
    bounds = [(c * per, min((c + 1) * per, n_rows)) for c in range(N_CORES)]
    r_max = max(e - s for s, e in bounds)
    r_pad = ((r_max + HALF - 1) // HALF) * HALF

    xb = x.astype(bf16)
    hb = h.astype(bf16)
    in_maps = []
    for s, e in bounds:
        xs = np.zeros((D, r_pad), bf16)
        hs = np.zeros((D, r_pad), bf16)
        xs[:, : e - s] = xb[s:e].T
        hs[:, : e - s] = hb[s:e].T
        im = {"xt": xs, "ht": hs, "wts": wts, "biases": biases}
        if cfg:
            im["wts8"] = (wt * W_SCALE).astype(fp8)
        if need_xh8:
            x8 = np.zeros((D, r_pad), fp8)
            h8 = np.zeros((D, r_pad), fp8)
            x8[:, : e - s] = xs[:, : e - s].astype(fp8)
            h8[:, : e - s] = hs[:, : e - s].astype(fp8)
            im["xt8"] = x8
            im["ht8"] = h8
        in_maps.append(im)

    nc = _get_nc(r_pad)
    res = bass_utils.run_bass_kernel_spmd(nc, in_maps, core_ids=list(range(N_CORES)))

    out = np.empty((n_rows, D), np.float32)
    for c, (s, e) in enumerate(bounds):
        out[s:e] = res.results[c]["y"][:, : e - s].T.astype(np.float32)
    return out
